# revision 4
# baseline (speedup 1.0000x reference)
"""FSUMGU cell on 8 Trainium2 NeuronCores — transposed-space formulation.

Math (per reference):
    zf = [hx, x] @ w_f.T + b_f
    fg = (zf + 1) / 2
    fgx = fg * hx
    ng = [fgx, x] @ w_n.T + b_n
    hy = (1 - fg) * ng + fgx

Sharding: 2 batch-halves (r) x 4 hidden-quarters (c); core id = r*4 + c.

Everything on-core is computed in TRANSPOSED space (hy^T[h, b]), which
makes every matmul operand naturally k-major:
    zf^T[h, b]  = sum_k wfT[k, h] * actT[k, b]      (stationary wfT tile)
    fg^T        = 0.5*zf^T + bfp[h]   (per-partition bias, scalar engine)
    fgx^T[h, b] = fg^T * hx^T[h, b]   (elementwise — NO PE transpose)
    ng^T, hy^T  analogous.
The host pre-transposes (and pre-casts to bf16) hx/input/w_f/w_n in
numpy, so the tensor engine runs ONLY the 512 real GEMM matmuls.

The only cross-core dependency is ng's contraction over the full hidden
dim of fgx: four small AllGathers of fgx^T column-chunks over each
4-core row group, hidden under phase-1/phase-2 compute (phase 2 runs
both b-blocks' CC-independent input-half contractions first).

Matmuls are emitted in PSUM-bank pairs sharing the same moving operand
so consecutive matmuls alternate banks, improving LDWEIGHTS overlap.
"""
import sys

sys.path.insert(0, "/opt/trn_rl_repo")

import numpy as np
import ml_dtypes
import concourse.tile as tile
from concourse import bacc, mybir
from concourse.bass_utils import run_bass_kernel_spmd

F32 = mybir.dt.float32
BF16 = mybir.dt.bfloat16
IDENT = mybir.ActivationFunctionType.Identity
MULT = mybir.AluOpType.mult
ADD = mybir.AluOpType.add

B, H, I = 2048, 2048, 2048
R, C = 2, 4
BL = B // R            # 1024 batch rows per core
HC = H // C            # 512 output features per core
BB = BL // 2           # 512 batch cols per b-block
CW = 256               # collective chunk width (b cols)
NKH = H // 128         # 16 k-tiles in the hx / fgx half
NKI = I // 128         # 16 k-tiles in the input half
NK = NKH + NKI         # 32 k-tiles total contraction
NA = HC // 128         # 4 hidden tiles per core slice
NWARM = 12             # HAM warm-up matmuls at kernel start

_NC_CACHE = None


def build():
    nc = bacc.Bacc(None, target_bir_lowering=False, debug=False)
    d_hxT = nc.dram_tensor("hxT", [H, BL], BF16, kind="ExternalInput").ap()
    d_inpT = nc.dram_tensor("inpT", [I, BL], BF16, kind="ExternalInput").ap()
    d_hxcT = nc.dram_tensor("hxcT", [HC, BL], BF16, kind="ExternalInput").ap()
    d_wfT = nc.dram_tensor("wfT", [H + I, HC], BF16, kind="ExternalInput").ap()
    d_wnT = nc.dram_tensor("wnT", [H + I, HC], BF16, kind="ExternalInput").ap()
    d_bf = nc.dram_tensor("bf", [128, NA], F32, kind="ExternalInput").ap()
    d_bn = nc.dram_tensor("bn", [128, NA], F32, kind="ExternalInput").ap()
    d_hyT = nc.dram_tensor("hyT", [HC, BL], F32, kind="ExternalOutput").ap()

    def kmaj(dram_ap, r0, nt, c0, ncols):
        """[nt*128, ncols] DRAM slab -> [128, nt, ncols] k-major AP."""
        return dram_ap[r0:r0 + nt * 128, c0:c0 + ncols].rearrange(
            "(t p) b -> p t b", p=128)

    with tile.TileContext(nc) as tc:
        with (
            tc.tile_pool(name="const", bufs=1) as const,
            tc.tile_pool(name="wf", bufs=1) as wfp,
            tc.tile_pool(name="wn", bufs=1) as wnp,
            tc.tile_pool(name="act", bufs=1) as actp,
            tc.tile_pool(name="gat", bufs=1) as gatp,
            tc.tile_pool(name="pers", bufs=1) as pers,
            tc.tile_pool(name="fgt", bufs=3) as fgtp,
            tc.tile_pool(name="scr", bufs=3) as scr,
            tc.tile_pool(name="outp", bufs=3) as outp,
            tc.tile_pool(name="dram", bufs=1, space="DRAM") as dram,
            tc.tile_pool(name="ps", bufs=8, space="PSUM") as ps,
        ):
            # ---- HAM warm-up: keep the PE counted busy while DMAs land
            wm = const.tile([128, 512], BF16, tag="wm")
            nc.vector.memset(wm[:], 0.0009765625)
            psw = ps.tile([128, 512], F32, tag="acc", name="psw")
            for i in range(NWARM):
                nc.tensor.matmul(psw[:], wm[:, :128], wm[:],
                                 start=(i == 0), stop=(i == NWARM - 1))

            # ---- persistent SBUF tensors
            s_wf = wfp.tile([128, NK, HC], BF16, tag="wf")
            s_wn = wnp.tile([128, NK, HC], BF16, tag="wn")
            s_hx = [actp.tile([128, NKH, BB], BF16, tag=f"hx{b}", name=f"s_hx{b}")
                    for b in range(2)]
            s_inp = [actp.tile([128, NKI, BB], BF16, tag=f"in{b}", name=f"s_inp{b}")
                     for b in range(2)]
            s_gat = [gatp.tile([128, NKH, BB], BF16, tag=f"gat{b}", name=f"s_gat{b}")
                     for b in range(2)]
            s_hxc = pers.tile([128, NA, BL], BF16, tag="hxc")
            s_fgx = pers.tile([128, NA, BL], BF16, tag="fgx")
            s_omf = pers.tile([128, NA, BL], BF16, tag="omf")

            # DRAM bounce buffers: 4 collective chunks (b-block x col-half)
            cc_in = [dram.tile([HC, CW], BF16, name=f"cc_in{j}") for j in range(4)]
            cc_out = [dram.tile([C, HC, CW], BF16, name=f"cc_out{j}")
                      for j in range(4)]

            # ---- gpsimd ring: biases, hxc, inp0, then w_n
            bfr = const.tile([128, NA], F32, tag="bfr")
            bnr = const.tile([128, NA], F32, tag="bnr")
            nc.gpsimd.dma_start(bfr[:], d_bf[:])
            nc.gpsimd.dma_start(bnr[:], d_bn[:])
            nc.gpsimd.dma_start(s_hxc[:], kmaj(d_hxcT, 0, NA, 0, BL))
            for g in range(4):
                nc.gpsimd.dma_start(s_inp[0][:, g * 4:(g + 1) * 4, :],
                                    kmaj(d_inpT, g * 512, 4, 0, BB))
            bfp = const.tile([128, NA], F32, tag="bfp")
            bfm = const.tile([128, NA], F32, tag="bfm")
            nc.vector.tensor_scalar(bfp[:], bfr[:], 0.5, 0.5, MULT, ADD)
            nc.vector.tensor_scalar(bfm[:], bfr[:], -0.5, 0.5, MULT, ADD)
            # w_n: input-half first (needed at phase-2 start), then hx-half
            for g in range(4):
                nc.gpsimd.dma_start(s_wn[:, 16 + g * 4:16 + (g + 1) * 4, :],
                                    kmaj(d_wnT, 2048 + g * 512, 4, 0, HC))
            for g in range(4):
                nc.gpsimd.dma_start(s_wn[:, g * 4:(g + 1) * 4, :],
                                    kmaj(d_wnT, g * 512, 4, 0, HC))

            # ---- sync ring: wf x hx0 interleaved (first chunks small so the
            # PE can start early), then hx1 x inp1
            nc.sync.dma_start(s_wf[:, 0:2, :], kmaj(d_wfT, 0, 2, 0, HC))
            nc.sync.dma_start(s_hx[0][:, 0:2, :], kmaj(d_hxT, 0, 2, 0, BB))
            nc.sync.dma_start(s_wf[:, 2:4, :], kmaj(d_wfT, 256, 2, 0, HC))
            nc.sync.dma_start(s_hx[0][:, 2:4, :], kmaj(d_hxT, 256, 2, 0, BB))
            for g in range(1, 4):
                nc.sync.dma_start(s_wf[:, g * 4:(g + 1) * 4, :],
                                  kmaj(d_wfT, g * 512, 4, 0, HC))
                nc.sync.dma_start(s_hx[0][:, g * 4:(g + 1) * 4, :],
                                  kmaj(d_hxT, g * 512, 4, 0, BB))
            for g in range(4):
                nc.sync.dma_start(s_wf[:, 16 + g * 4:16 + (g + 1) * 4, :],
                                  kmaj(d_wfT, 2048 + g * 512, 4, 0, HC))
            for g in range(4):
                nc.sync.dma_start(s_hx[1][:, g * 4:(g + 1) * 4, :],
                                  kmaj(d_hxT, g * 512, 4, BB, BB))
                nc.sync.dma_start(s_inp[1][:, g * 4:(g + 1) * 4, :],
                                  kmaj(d_inpT, g * 512, 4, BB, BB))

            def fire_gather(j):
                nc.gpsimd.collective_compute(
                    "AllGather",
                    mybir.AluOpType.bypass,
                    replica_groups=[[0, 1, 2, 3], [4, 5, 6, 7]],
                    ins=[cc_in[j].opt()],
                    outs=[cc_out[j].opt()],
                )
                bb, half = j // 2, j % 2
                for q in range(4):
                    nc.gpsimd.dma_start(
                        s_gat[bb][:, q * 4:(q + 1) * 4, half * CW:(half + 1) * CW],
                        cc_out[j][q].rearrange("(t p) b -> p t b", p=128))

            # ---- phase 1: zf^T -> fg^T / (1-fg)^T / fgx^T
            # a-tiles in PSUM-bank pairs sharing the moving operand
            for bb in range(2):
                bcol = slice(bb * BB, (bb + 1) * BB)
                for ap in (0, 2):
                    acc = [ps.tile([128, HC], F32, tag="acc", name=f"p1acc{bb}{ap}{u}")
                           for u in range(2)]
                    for kt in range(NK):
                        rhs = (s_hx[bb][:, kt, :] if kt < NKH
                               else s_inp[bb][:, kt - NKH, :])
                        for u in range(2):
                            a = ap + u
                            nc.tensor.matmul(acc[u][:],
                                             s_wf[:, kt, a * 128:(a + 1) * 128],
                                             rhs,
                                             start=(kt == 0), stop=(kt == NK - 1))
                    for u in range(2):
                        a = ap + u
                        fgt = fgtp.tile([128, BB], BF16, tag="fgt")
                        nc.scalar.activation(fgt[:], acc[u][:], IDENT,
                                             bias=bfp[:, a:a + 1], scale=0.5)
                        nc.scalar.activation(s_omf[:, a, bcol], acc[u][:], IDENT,
                                             bias=bfm[:, a:a + 1], scale=-0.5)
                        nc.vector.tensor_mul(s_fgx[:, a, bcol], fgt[:],
                                             s_hxc[:, a, bcol])
                        for half in range(2):
                            j = bb * 2 + half
                            nc.scalar.dma_start(
                                cc_in[j][a * 128:(a + 1) * 128, :],
                                s_fgx[:, a, bb * BB + half * CW:
                                      bb * BB + (half + 1) * CW])
                fire_gather(bb * 2)
                fire_gather(bb * 2 + 1)

            # ---- phase 2: ng^T, hy^T. Both b-blocks' input-half contractions
            # run first (CC-independent) so all four AllGathers stay hidden.
            accs = {}
            for bb in range(2):
                for ap in (0, 2):
                    acc = [ps.tile([128, HC], F32, tag="acc", name=f"p2acc{bb}{ap}{u}")
                           for u in range(2)]
                    accs[(bb, ap)] = acc
                    for i in range(NKI):
                        for u in range(2):
                            a = ap + u
                            nc.tensor.matmul(acc[u][:],
                                             s_wn[:, NKH + i, a * 128:(a + 1) * 128],
                                             s_inp[bb][:, i, :],
                                             start=(i == 0), stop=False)
            for bb in range(2):
                bcol = slice(bb * BB, (bb + 1) * BB)
                for ap in (0, 2):
                    acc = accs[(bb, ap)]
                    for kt in range(NKH):
                        for u in range(2):
                            a = ap + u
                            nc.tensor.matmul(acc[u][:],
                                             s_wn[:, kt, a * 128:(a + 1) * 128],
                                             s_gat[bb][:, kt, :],
                                             start=False, stop=(kt == NKH - 1))
                    for u in range(2):
                        a = ap + u
                        t = scr.tile([128, BB], F32, tag="t")
                        nc.vector.scalar_tensor_tensor(
                            t[:], acc[u][:], bnr[:, a:a + 1], s_omf[:, a, bcol],
                            ADD, MULT)
                        o = outp.tile([128, BB], F32, tag="o")
                        nc.vector.tensor_add(o[:], t[:], s_fgx[:, a, bcol])
                        nc.scalar.dma_start(
                            d_hyT[a * 128:(a + 1) * 128, bb * BB:(bb + 1) * BB],
                            o[:])

    nc.finalize()
    return nc


def _get_nc():
    global _NC_CACHE
    if _NC_CACHE is None:
        _NC_CACHE = build()
    return _NC_CACHE


def prepare_in_maps(input, hx, w_f, b_f, w_n, b_n):
    bf16 = ml_dtypes.bfloat16
    hxT_r, inpT_r = [], []
    for r in range(R):
        hxT_r.append(np.ascontiguousarray(
            hx[r * BL:(r + 1) * BL, :].T.astype(bf16)))
        inpT_r.append(np.ascontiguousarray(
            input[r * BL:(r + 1) * BL, :].T.astype(bf16)))
    wfT_c, wnT_c, bf_c, bn_c = [], [], [], []
    for c in range(C):
        wfT_c.append(np.ascontiguousarray(
            w_f[c * HC:(c + 1) * HC, :].T.astype(bf16)))
        wnT_c.append(np.ascontiguousarray(
            w_n[c * HC:(c + 1) * HC, :].T.astype(bf16)))
        bf_c.append(np.ascontiguousarray(
            b_f[c * HC:(c + 1) * HC].reshape(NA, 128).T.astype(np.float32)))
        bn_c.append(np.ascontiguousarray(
            b_n[c * HC:(c + 1) * HC].reshape(NA, 128).T.astype(np.float32)))
    in_maps = []
    for core in range(R * C):
        r, c = core // C, core % C
        in_maps.append({
            "hxT": hxT_r[r],
            "inpT": inpT_r[r],
            "hxcT": np.ascontiguousarray(hxT_r[r][c * HC:(c + 1) * HC, :]),
            "wfT": wfT_c[c],
            "wnT": wnT_c[c],
            "bf": bf_c[c],
            "bn": bn_c[c],
        })
    return in_maps


def assemble_output(results):
    rows = []
    for r in range(R):
        rows.append(np.concatenate(
            [np.asarray(results[r * C + c]["hyT"], dtype=np.float32).T
             for c in range(C)], axis=1))
    return np.ascontiguousarray(np.concatenate(rows, axis=0))


def kernel(input, hx, w_f, b_f, w_n, b_n, **_ignored):
    input = np.asarray(input, dtype=np.float32)
    hx = np.asarray(hx, dtype=np.float32)
    w_f = np.asarray(w_f, dtype=np.float32)
    b_f = np.asarray(b_f, dtype=np.float32)
    w_n = np.asarray(w_n, dtype=np.float32)
    b_n = np.asarray(b_n, dtype=np.float32)

    nc = _get_nc()
    in_maps = prepare_in_maps(input, hx, w_f, b_f, w_n, b_n)
    res = run_bass_kernel_spmd(nc, in_maps, list(range(R * C)))
    return assemble_output(res.results)


if __name__ == "__main__":
    rng = np.random.default_rng(0)
    inputs = {
        "input": rng.uniform(-1, 1, (B, I)).astype(np.float32),
        "hx": rng.uniform(-1, 1, (B, H)).astype(np.float32),
        "w_f": (rng.standard_normal((H, H + I)) / np.sqrt(H + I)).astype(np.float32),
        "b_f": (rng.standard_normal(H) / np.sqrt(H + I)).astype(np.float32),
        "w_n": (rng.standard_normal((H, H + I)) / np.sqrt(H + I)).astype(np.float32),
        "b_n": (rng.standard_normal(H) / np.sqrt(H + I)).astype(np.float32),
    }
    out = kernel(**inputs)
    x64 = {k: v.astype(np.float64) for k, v in inputs.items()}
    cat = np.concatenate([x64["hx"], x64["input"]], axis=1)
    fg = (cat @ x64["w_f"].T + x64["b_f"] + 1.0) * 0.5
    fgx = fg * x64["hx"]
    ng = np.concatenate([fgx, x64["input"]], axis=1) @ x64["w_n"].T + x64["b_n"]
    exp = (1.0 - fg) * ng + fgx
    err = np.abs(out - exp).max() / np.abs(exp).max()
    print("rel err:", err)


# revision 5
# speedup vs baseline: 1.1134x; 1.1134x over previous
"""FSUMGU cell on 8 Trainium2 NeuronCores — transposed-space formulation.

Math (per reference):
    zf = [hx, x] @ w_f.T + b_f
    fg = (zf + 1) / 2
    fgx = fg * hx
    ng = [fgx, x] @ w_n.T + b_n
    hy = (1 - fg) * ng + fgx

Sharding: 2 batch-halves (r) x 4 hidden-quarters (c); core id = r*4 + c.

Everything on-core is computed in TRANSPOSED space (hy^T[h, b]), which
makes every matmul operand naturally k-major:
    zf^T[h, b]  = sum_k wfT[k, h] * actT[k, b]      (stationary wfT tile)
    fg^T        = 0.5*zf^T + bfp[h]   (per-partition bias, scalar engine)
    fgx^T[h, b] = fg^T * hx^T[h, b]   (elementwise — NO PE transpose)
    ng^T, hy^T  analogous.
The host pre-transposes (and pre-casts to bf16) hx/input/w_f/w_n in
numpy, so the tensor engine runs ONLY the 512 real GEMM matmuls.

The only cross-core dependency is ng's contraction over the full hidden
dim of fgx: one AllGather of fgx^T per batch-half over each 4-core row
group. The CC stream carries a fixed ~35us init barrier, then the two
gathers back-to-back (~26us each at ~60 GB/s); phase 2 therefore runs
BOTH b-blocks' CC-independent input-half contractions first, pushing
the second gather's need-time past its completion. Both gather triggers
are issued adjacently on the gpsimd queue (the gathered read-backs live
on the sync ring) so neither trigger is queue-blocked.

Phase 1 contracts k-half by k-half (4 open PSUM banks) so the early DMA
demand is half of what a full-k sweep needs.
"""
import sys

sys.path.insert(0, "/opt/trn_rl_repo")

import numpy as np
import ml_dtypes
import concourse.tile as tile
from concourse import bacc, mybir
from concourse.bass_utils import run_bass_kernel_spmd

F32 = mybir.dt.float32
BF16 = mybir.dt.bfloat16
IDENT = mybir.ActivationFunctionType.Identity
MULT = mybir.AluOpType.mult
ADD = mybir.AluOpType.add

B, H, I = 2048, 2048, 2048
R, C = 2, 4
BL = B // R            # 1024 batch rows per core
HC = H // C            # 512 output features per core
BB = BL // 2           # 512 batch cols per b-block
NKH = H // 128         # 16 k-tiles in the hx / fgx half
NKI = I // 128         # 16 k-tiles in the input half
NK = NKH + NKI         # 32 k-tiles total contraction
NA = HC // 128         # 4 hidden tiles per core slice
NWARM = 10             # HAM warm-up matmuls at kernel start

_NC_CACHE = None


def build():
    nc = bacc.Bacc(None, target_bir_lowering=False, debug=False)
    d_hxT = nc.dram_tensor("hxT", [H, BL], BF16, kind="ExternalInput").ap()
    d_inpT = nc.dram_tensor("inpT", [I, BL], BF16, kind="ExternalInput").ap()
    d_hxcT = nc.dram_tensor("hxcT", [HC, BL], BF16, kind="ExternalInput").ap()
    d_wfT = nc.dram_tensor("wfT", [H + I, HC], BF16, kind="ExternalInput").ap()
    d_wnT = nc.dram_tensor("wnT", [H + I, HC], BF16, kind="ExternalInput").ap()
    d_bf = nc.dram_tensor("bf", [128, NA], F32, kind="ExternalInput").ap()
    d_bn = nc.dram_tensor("bn", [128, NA], F32, kind="ExternalInput").ap()
    d_hyT = nc.dram_tensor("hyT", [HC, BL], F32, kind="ExternalOutput").ap()

    def kmaj(dram_ap, r0, nt, c0, ncols):
        """[nt*128, ncols] DRAM slab -> [128, nt, ncols] k-major AP."""
        return dram_ap[r0:r0 + nt * 128, c0:c0 + ncols].rearrange(
            "(t p) b -> p t b", p=128)

    with tile.TileContext(nc) as tc:
        with (
            tc.tile_pool(name="const", bufs=1) as const,
            tc.tile_pool(name="wf", bufs=1) as wfp,
            tc.tile_pool(name="wn", bufs=1) as wnp,
            tc.tile_pool(name="act", bufs=1) as actp,
            tc.tile_pool(name="gat", bufs=1) as gatp,
            tc.tile_pool(name="pers", bufs=1) as pers,
            tc.tile_pool(name="fgt", bufs=3) as fgtp,
            tc.tile_pool(name="scr", bufs=3) as scr,
            tc.tile_pool(name="outp", bufs=3) as outp,
            tc.tile_pool(name="dram", bufs=1, space="DRAM") as dram,
            tc.tile_pool(name="ps", bufs=8, space="PSUM") as ps,
        ):
            # ---- HAM warm-up: keep the PE counted busy while DMAs land
            wm = const.tile([128, 512], BF16, tag="wm")
            nc.vector.memset(wm[:], 0.0009765625)
            psw = ps.tile([128, 512], F32, tag="acc", name="psw")
            for i in range(NWARM):
                nc.tensor.matmul(psw[:], wm[:, :128], wm[:],
                                 start=(i == 0), stop=(i == NWARM - 1))

            # ---- persistent SBUF tensors
            s_wf = wfp.tile([128, NK, HC], BF16, tag="wf")
            s_wn = wnp.tile([128, NK, HC], BF16, tag="wn")
            s_hx = [actp.tile([128, NKH, BB], BF16, tag=f"hx{b}", name=f"s_hx{b}")
                    for b in range(2)]
            s_inp = [actp.tile([128, NKI, BB], BF16, tag=f"in{b}", name=f"s_inp{b}")
                     for b in range(2)]
            s_gat = [gatp.tile([128, NKH, BB], BF16, tag=f"gat{b}", name=f"s_gat{b}")
                     for b in range(2)]
            s_hxc = pers.tile([128, NA, BL], BF16, tag="hxc")
            s_fgx = pers.tile([128, NA, BL], BF16, tag="fgx")
            s_omf = pers.tile([128, NA, BL], BF16, tag="omf")

            # DRAM bounce buffers for the two per-b-block AllGathers
            cc_in = [dram.tile([HC, BB], BF16, name=f"cc_in{j}") for j in range(2)]
            cc_out = [dram.tile([C, HC, BB], BF16, name=f"cc_out{j}")
                      for j in range(2)]

            # ---- gpsimd ring: biases + hxc, then ONLY the gather triggers
            bfr = const.tile([128, NA], F32, tag="bfr")
            bnr = const.tile([128, NA], F32, tag="bnr")
            nc.gpsimd.dma_start(bfr[:], d_bf[:])
            nc.gpsimd.dma_start(bnr[:], d_bn[:])
            nc.gpsimd.dma_start(s_hxc[:], kmaj(d_hxcT, 0, NA, 0, BL))
            bfp = const.tile([128, NA], F32, tag="bfp")
            bfm = const.tile([128, NA], F32, tag="bfm")
            nc.vector.tensor_scalar(bfp[:], bfr[:], 0.5, 0.5, MULT, ADD)
            nc.vector.tensor_scalar(bfm[:], bfr[:], -0.5, 0.5, MULT, ADD)

            # ---- sync ring, FIFO = consumption order.
            # Phase 1 contracts k-half 0 first (wf kh0 x hx0), then k-half 1
            # (wf kh1 x inp0); first chunks split small for an early PE start.
            nc.sync.dma_start(s_wf[:, 0:2, :], kmaj(d_wfT, 0, 2, 0, HC))
            nc.sync.dma_start(s_hx[0][:, 0:2, :], kmaj(d_hxT, 0, 2, 0, BB))
            nc.sync.dma_start(s_wf[:, 2:4, :], kmaj(d_wfT, 256, 2, 0, HC))
            nc.sync.dma_start(s_hx[0][:, 2:4, :], kmaj(d_hxT, 256, 2, 0, BB))
            for g in range(1, 4):
                nc.sync.dma_start(s_wf[:, g * 4:(g + 1) * 4, :],
                                  kmaj(d_wfT, g * 512, 4, 0, HC))
                nc.sync.dma_start(s_hx[0][:, g * 4:(g + 1) * 4, :],
                                  kmaj(d_hxT, g * 512, 4, 0, BB))
            for g in range(4):
                nc.sync.dma_start(s_wf[:, 16 + g * 4:16 + (g + 1) * 4, :],
                                  kmaj(d_wfT, 2048 + g * 512, 4, 0, HC))
                nc.sync.dma_start(s_inp[0][:, g * 4:(g + 1) * 4, :],
                                  kmaj(d_inpT, g * 512, 4, 0, BB))
            for g in range(4):
                nc.sync.dma_start(s_hx[1][:, g * 4:(g + 1) * 4, :],
                                  kmaj(d_hxT, g * 512, 4, BB, BB))
                nc.sync.dma_start(s_inp[1][:, g * 4:(g + 1) * 4, :],
                                  kmaj(d_inpT, g * 512, 4, BB, BB))
            # w_n: input-half first (needed at phase-2 start), then hx-half
            for g in range(4):
                nc.sync.dma_start(s_wn[:, 16 + g * 4:16 + (g + 1) * 4, :],
                                  kmaj(d_wnT, 2048 + g * 512, 4, 0, HC))
            for g in range(4):
                nc.sync.dma_start(s_wn[:, g * 4:(g + 1) * 4, :],
                                  kmaj(d_wnT, g * 512, 4, 0, HC))

            # ---- phase 1: zf^T -> fg^T / (1-fg)^T / fgx^T
            # k-half sweeps with 4 open PSUM banks per b-block
            for bb in range(2):
                bcol = slice(bb * BB, (bb + 1) * BB)
                accs = [ps.tile([128, HC], F32, tag="acc", name=f"p1acc{bb}{a}")
                        for a in range(NA)]
                for a in range(NA):
                    for kt in range(NKH):
                        nc.tensor.matmul(accs[a][:],
                                         s_wf[:, kt, a * 128:(a + 1) * 128],
                                         s_hx[bb][:, kt, :],
                                         start=(kt == 0), stop=False)
                for a in range(NA):
                    for i in range(NKI):
                        nc.tensor.matmul(accs[a][:],
                                         s_wf[:, NKH + i, a * 128:(a + 1) * 128],
                                         s_inp[bb][:, i, :],
                                         start=False, stop=(i == NKI - 1))
                    fgt = fgtp.tile([128, BB], BF16, tag="fgt")
                    nc.scalar.activation(fgt[:], accs[a][:], IDENT,
                                         bias=bfp[:, a:a + 1], scale=0.5)
                    nc.scalar.activation(s_omf[:, a, bcol], accs[a][:], IDENT,
                                         bias=bfm[:, a:a + 1], scale=-0.5)
                    nc.vector.tensor_mul(s_fgx[:, a, bcol], fgt[:],
                                         s_hxc[:, a, bcol])
                    nc.scalar.dma_start(cc_in[bb][a * 128:(a + 1) * 128, :],
                                        s_fgx[:, a, bcol])

            # ---- both gather triggers adjacent on the gpsimd queue
            for j in range(2):
                nc.gpsimd.collective_compute(
                    "AllGather",
                    mybir.AluOpType.bypass,
                    replica_groups=[[0, 1, 2, 3], [4, 5, 6, 7]],
                    ins=[cc_in[j].opt()],
                    outs=[cc_out[j].opt()],
                )
            # gathered fgx^T read-backs on the (now idle) sync ring
            for j in range(2):
                for q in range(4):
                    nc.sync.dma_start(
                        s_gat[j][:, q * 4:(q + 1) * 4, :],
                        cc_out[j][q].rearrange("(t p) b -> p t b", p=128))

            # ---- phase 2: ng^T, hy^T. Both b-blocks' input-half contractions
            # run first (CC-independent) so the AllGathers stay hidden.
            accs = {}
            for bb in range(2):
                for a in range(NA):
                    acc = ps.tile([128, HC], F32, tag="acc", name=f"p2acc{bb}{a}")
                    accs[(bb, a)] = acc
                    for i in range(NKI):
                        nc.tensor.matmul(acc[:],
                                         s_wn[:, NKH + i, a * 128:(a + 1) * 128],
                                         s_inp[bb][:, i, :],
                                         start=(i == 0), stop=False)
            for bb in range(2):
                bcol = slice(bb * BB, (bb + 1) * BB)
                for a in range(NA):
                    acc = accs[(bb, a)]
                    for kt in range(NKH):
                        nc.tensor.matmul(acc[:],
                                         s_wn[:, kt, a * 128:(a + 1) * 128],
                                         s_gat[bb][:, kt, :],
                                         start=False, stop=(kt == NKH - 1))
                    t = scr.tile([128, BB], F32, tag="t")
                    nc.vector.scalar_tensor_tensor(
                        t[:], acc[:], bnr[:, a:a + 1], s_omf[:, a, bcol],
                        ADD, MULT)
                    o = outp.tile([128, BB], F32, tag="o")
                    nc.vector.tensor_add(o[:], t[:], s_fgx[:, a, bcol])
                    nc.scalar.dma_start(
                        d_hyT[a * 128:(a + 1) * 128, bb * BB:(bb + 1) * BB],
                        o[:])

    nc.finalize()
    return nc


def _get_nc():
    global _NC_CACHE
    if _NC_CACHE is None:
        _NC_CACHE = build()
    return _NC_CACHE


def prepare_in_maps(input, hx, w_f, b_f, w_n, b_n):
    bf16 = ml_dtypes.bfloat16
    hxT_r, inpT_r = [], []
    for r in range(R):
        hxT_r.append(np.ascontiguousarray(
            hx[r * BL:(r + 1) * BL, :].T.astype(bf16)))
        inpT_r.append(np.ascontiguousarray(
            input[r * BL:(r + 1) * BL, :].T.astype(bf16)))
    wfT_c, wnT_c, bf_c, bn_c = [], [], [], []
    for c in range(C):
        wfT_c.append(np.ascontiguousarray(
            w_f[c * HC:(c + 1) * HC, :].T.astype(bf16)))
        wnT_c.append(np.ascontiguousarray(
            w_n[c * HC:(c + 1) * HC, :].T.astype(bf16)))
        bf_c.append(np.ascontiguousarray(
            b_f[c * HC:(c + 1) * HC].reshape(NA, 128).T.astype(np.float32)))
        bn_c.append(np.ascontiguousarray(
            b_n[c * HC:(c + 1) * HC].reshape(NA, 128).T.astype(np.float32)))
    in_maps = []
    for core in range(R * C):
        r, c = core // C, core % C
        in_maps.append({
            "hxT": hxT_r[r],
            "inpT": inpT_r[r],
            "hxcT": np.ascontiguousarray(hxT_r[r][c * HC:(c + 1) * HC, :]),
            "wfT": wfT_c[c],
            "wnT": wnT_c[c],
            "bf": bf_c[c],
            "bn": bn_c[c],
        })
    return in_maps


def assemble_output(results):
    rows = []
    for r in range(R):
        rows.append(np.concatenate(
            [np.asarray(results[r * C + c]["hyT"], dtype=np.float32).T
             for c in range(C)], axis=1))
    return np.ascontiguousarray(np.concatenate(rows, axis=0))


def kernel(input, hx, w_f, b_f, w_n, b_n, **_ignored):
    input = np.asarray(input, dtype=np.float32)
    hx = np.asarray(hx, dtype=np.float32)
    w_f = np.asarray(w_f, dtype=np.float32)
    b_f = np.asarray(b_f, dtype=np.float32)
    w_n = np.asarray(w_n, dtype=np.float32)
    b_n = np.asarray(b_n, dtype=np.float32)

    nc = _get_nc()
    in_maps = prepare_in_maps(input, hx, w_f, b_f, w_n, b_n)
    res = run_bass_kernel_spmd(nc, in_maps, list(range(R * C)))
    return assemble_output(res.results)


if __name__ == "__main__":
    rng = np.random.default_rng(0)
    inputs = {
        "input": rng.uniform(-1, 1, (B, I)).astype(np.float32),
        "hx": rng.uniform(-1, 1, (B, H)).astype(np.float32),
        "w_f": (rng.standard_normal((H, H + I)) / np.sqrt(H + I)).astype(np.float32),
        "b_f": (rng.standard_normal(H) / np.sqrt(H + I)).astype(np.float32),
        "w_n": (rng.standard_normal((H, H + I)) / np.sqrt(H + I)).astype(np.float32),
        "b_n": (rng.standard_normal(H) / np.sqrt(H + I)).astype(np.float32),
    }
    out = kernel(**inputs)
    x64 = {k: v.astype(np.float64) for k, v in inputs.items()}
    cat = np.concatenate([x64["hx"], x64["input"]], axis=1)
    fg = (cat @ x64["w_f"].T + x64["b_f"] + 1.0) * 0.5
    fgx = fg * x64["hx"]
    ng = np.concatenate([fgx, x64["input"]], axis=1) @ x64["w_n"].T + x64["b_n"]
    exp = (1.0 - fg) * ng + fgx
    err = np.abs(out - exp).max() / np.abs(exp).max()
    print("rel err:", err)


# revision 9
# speedup vs baseline: 1.1135x; 1.0000x over previous
"""FSUMGU cell on 8 Trainium2 NeuronCores — pure data-parallel variant.

Each core owns 256 batch rows and computes the ENTIRE cell for them:
no collectives at all (the AllGather of the tensor-parallel variant
costs a ~40-70us CC-stream init barrier plus 2x ~30us serialized
gathers, a large fraction of which lands on the critical path).

Layout is [batch, hidden] ("row" orientation):
    zf[b, h] = sum_k catT[k, b].T @ wfT[k, h]   (stationary actT tile,
                                                 moving 512-wide weight cols)
    fg/omf/fgx elementwise on [128b, 512h] tiles (vector engine,
                                                  broadcast bias rows)
    fgx^T for GEMM2's hidden contraction comes from a DMA XBAR
    transpose (zero tensor-engine cost).

Weights stream through a single [128, 32, 2048] bf16 arena per matrix,
time-shared k-half by k-half: phase 1 consumes wf kh0 then kh1; wn's
input-half loads into wf-kh0's slot once that half is consumed, and
wn's fgx-half into wf-kh1's slot during phase 2's input-half.

Stationary tiles are shared across the four 512-col h-blocks
(consecutive matmuls with identical lhsT), amortizing LDWEIGHTS.

Bias rows are broadcast to 128 partitions with ones-matmuls that
double as the tensor-engine HAM warm-up.
"""
import sys

sys.path.insert(0, "/opt/trn_rl_repo")

import numpy as np
import ml_dtypes
import concourse.tile as tile
from concourse import bacc, mybir
from concourse.bass_utils import run_bass_kernel_spmd

F32 = mybir.dt.float32
BF16 = mybir.dt.bfloat16
MULT = mybir.AluOpType.mult
ADD = mybir.AluOpType.add

B, H, I = 2048, 2048, 2048
NCORES = 8
BL = B // NCORES       # 256 batch rows per core
NBT = BL // 128        # 2 batch tiles
NHB = H // 512         # 4 hidden 512-col blocks
NKH = H // 128         # 16 k-tiles, hx/fgx half
NKI = I // 128         # 16 k-tiles, input half
NK = NKH + NKI         # 32
NWARM = 4              # pure warm-up matmuls before the bias broadcasts

_NC_CACHE = None


def build():
    nc = bacc.Bacc(None, target_bir_lowering=False, debug=False)
    d_actT = nc.dram_tensor("actT", [H + I, BL], BF16, kind="ExternalInput").ap()
    d_hxr = nc.dram_tensor("hxr", [BL, H], BF16, kind="ExternalInput").ap()
    d_wfT = nc.dram_tensor("wfT", [H + I, H], BF16, kind="ExternalInput").ap()
    d_wnT = nc.dram_tensor("wnT", [H + I, H], BF16, kind="ExternalInput").ap()
    d_bf = nc.dram_tensor("bf", [1, H], F32, kind="ExternalInput").ap()
    d_bn = nc.dram_tensor("bn", [1, H], F32, kind="ExternalInput").ap()
    d_hy = nc.dram_tensor("hy", [BL, H], F32, kind="ExternalOutput").ap()

    def kmaj(dram_ap, r0, nt, c0, ncols):
        """[nt*128, ncols] DRAM slab -> [128, nt, ncols] k-major AP."""
        return dram_ap[r0:r0 + nt * 128, c0:c0 + ncols].rearrange(
            "(t p) b -> p t b", p=128)

    with tile.TileContext(nc) as tc:
        with (
            tc.tile_pool(name="const", bufs=1) as const,
            tc.tile_pool(name="warena", bufs=1) as warena,
            tc.tile_pool(name="act", bufs=1) as actp,
            tc.tile_pool(name="pers", bufs=1) as pers,
            tc.tile_pool(name="fgt", bufs=3) as fgtp,
            tc.tile_pool(name="scr", bufs=3) as scr,
            tc.tile_pool(name="ps", bufs=8, space="PSUM") as ps,
        ):
            # ---- persistent SBUF tensors
            # weight arena: two k-half slots, time-shared wf -> wn
            wA = warena.tile([128, NKH, H], BF16, tag="wA")   # wf kh0 -> wn kh1(inp)
            wB = warena.tile([128, NKH, H], BF16, tag="wB")   # wf kh1 -> wn kh0(fgx)
            s_act = actp.tile([128, NK, BL], BF16, tag="actT")     # [hx; inp]^T
            s_hxr = pers.tile([128, NBT, H], BF16, tag="hxr")
            s_fgx = pers.tile([128, NBT, H], BF16, tag="fgx")
            s_omf = pers.tile([128, NBT, H], BF16, tag="omf")
            bias_bc = const.tile([128, 2, H], BF16, tag="biasbc")  # bfp, bn

            # ---- small loads (gpsimd): bias rows, hx row-layout copy
            rowf = const.tile([1, H], F32, tag="rowf")
            nc.gpsimd.dma_start(rowf[:], d_bf[:])
            nc.gpsimd.dma_start(s_hxr[:], d_hxr.rearrange("(t p) h -> p t h", p=128))

            # ---- bulk loads. sync ring: wf-kh0 x act-kh0 (kh0-critical),
            # then act-kh1, then its share of wn. scalar ring: wf-kh1 early
            # (needed only from kh1) + its share of wn + transposes + hy.
            # kt-pair interleave across BOTH HWDGE rings in strict need
            # order: [wA + act-kh0] then [wB + act-kh1]. Aggregate HBM BW
            # (~300 GB/s) is the limit; both rings must carry the critical
            # stream, earliest k-tiles first.
            for j in range(8):
                eng = nc.sync if j % 2 == 0 else nc.scalar
                kt = j * 2
                eng.dma_start(wA[:, kt:kt + 2, :], kmaj(d_wfT, kt * 128, 2, 0, H))
                eng.dma_start(s_act[:, kt:kt + 2, :],
                              kmaj(d_actT, kt * 128, 2, 0, BL))
            for j in range(8):
                eng = nc.sync if j % 2 == 0 else nc.scalar
                kt = j * 2
                eng.dma_start(s_act[:, 16 + kt:16 + kt + 2, :],
                              kmaj(d_actT, 2048 + kt * 128, 2, 0, BL))
                eng.dma_start(wB[:, kt:kt + 2, :],
                              kmaj(d_wfT, 2048 + kt * 128, 2, 0, H))

            # ---- bias prep: bfp=(b_f+1)/2 and bn as bf16 rows
            row16 = const.tile([1, 2, H], BF16, tag="row16")
            nc.vector.tensor_scalar(row16[:, 0, :], rowf[:],
                                    0.5, 0.5, MULT, ADD)
            nc.gpsimd.dma_start(rowf[:], d_bn[:])
            nc.vector.tensor_copy(row16[:, 1, :], rowf[:])
            ones = const.tile([1, 128], BF16, tag="ones")
            nc.vector.memset(ones[:], 1.0)
            wm = const.tile([128, 512], BF16, tag="wm")
            nc.vector.memset(wm[:], 0.0009765625)

            # ---- HAM warm-up + bias broadcast matmuls (also PE work)
            psw = ps.tile([128, 512], F32, tag="acc", name="psw")
            for i in range(NWARM):
                nc.tensor.matmul(psw[:], wm[:, :128], wm[:],
                                 start=(i == 0), stop=(i == NWARM - 1))
            for bi in range(2):
                for hb in range(NHB):
                    pb = ps.tile([128, 512], F32, tag="acc", name=f"pb{bi}{hb}")
                    nc.tensor.matmul(pb[:], ones[:],
                                     row16[:, bi, hb * 512:(hb + 1) * 512],
                                     start=True, stop=True)
                    nc.vector.tensor_copy(bias_bc[:, bi, hb * 512:(hb + 1) * 512],
                                          pb[:])
            bfp_bc = bias_bc[:, 0, :]
            bn_bc = bias_bc[:, 1, :]

            # ---- phase 1: zf -> fg/omf/fgx, k-half by k-half.
            # Stationary actT tile shared across the four h-blocks.
            accs1 = {}
            for bt in range(NBT):
                for hb in range(NHB):
                    accs1[(bt, hb)] = ps.tile([128, 512], F32, tag="acc",
                                              name=f"p1acc{bt}{hb}")
            for kh, warr in ((0, wA), (1, wB)):
                for bt in range(NBT):
                    for kt in range(NKH):
                        for hb in range(NHB):
                            mm = nc.tensor.matmul(
                                accs1[(bt, hb)][:],
                                s_act[:, kh * NKH + kt, bt * 128:(bt + 1) * 128],
                                warr[:, kt, hb * 512:(hb + 1) * 512],
                                start=(kh == 0 and kt == 0),
                                stop=(kh == 1 and kt == NKH - 1))
                            if hb > 0 and not (kh == 0 and kt == 0):
                                mm.ins.ldweights = False
            # ---- wn input-half loads into wf-kh0's slot (wA now dead)
            for j in range(8):
                eng = nc.sync if j % 2 == 0 else nc.scalar
                kt = j * 2
                eng.dma_start(wA[:, kt:kt + 2, :],
                              kmaj(d_wnT, 2048 + kt * 128, 2, 0, H))
            for bt in range(NBT):
                for hb in range(NHB):
                    acc = accs1[(bt, hb)]
                    hcol = slice(hb * 512, (hb + 1) * 512)
                    fgt = fgtp.tile([128, 512], BF16, tag="fgt")
                    nc.vector.scalar_tensor_tensor(
                        fgt[:], acc[:], 0.5, bfp_bc[:, hcol], MULT, ADD)
                    nc.vector.tensor_scalar(s_omf[:, bt, hcol], fgt[:],
                                            -1.0, 1.0, MULT, ADD)
                    nc.vector.tensor_mul(s_fgx[:, bt, hcol], fgt[:],
                                         s_hxr[:, bt, hcol])
                # fgx^T via DMA XBAR transposes (scalar HWDGE ring),
                # aliased into s_act's now-dead hx half. Per-k-tile so every
                # destination is a contiguous [128, 128] segment (XBAR
                # transpose corrupts non-contiguous destinations).
                for kt in range(NKH):
                    eng = nc.sync if kt % 2 == 0 else nc.scalar
                    eng.dma_start_transpose(
                        s_act[:, kt, bt * 128:(bt + 1) * 128],
                        s_fgx[:, bt, kt * 128:(kt + 1) * 128])

            # ---- wn fgx-half loads into wf-kh1's slot (wB dead after ph1)
            for j in range(8):
                eng = nc.sync if j % 2 == 0 else nc.scalar
                kt = j * 2
                eng.dma_start(wB[:, kt:kt + 2, :],
                              kmaj(d_wnT, kt * 128, 2, 0, H))

            # ---- phase 2: ng, hy. Input-half contraction first.
            accs2 = {}
            for bt in range(NBT):
                for hb in range(NHB):
                    accs2[(bt, hb)] = ps.tile([128, 512], F32, tag="acc",
                                              name=f"p2acc{bt}{hb}")
            for bt in range(NBT):
                for kt in range(NKI):
                    for hb in range(NHB):
                        mm = nc.tensor.matmul(
                            accs2[(bt, hb)][:],
                            s_act[:, NKH + kt, bt * 128:(bt + 1) * 128],
                            wA[:, kt, hb * 512:(hb + 1) * 512],
                            start=(kt == 0), stop=False)
                        if hb > 0 and kt > 0:
                            mm.ins.ldweights = False
            for bt in range(NBT):
                for kt in range(NKH - 2):
                    for hb in range(NHB):
                        mm = nc.tensor.matmul(
                            accs2[(bt, hb)][:],
                            s_act[:, kt, bt * 128:(bt + 1) * 128],
                            wB[:, kt, hb * 512:(hb + 1) * 512],
                            start=False, stop=False)
                        if hb > 0:
                            mm.ins.ldweights = False
                for hb in range(NHB):
                    for kt in (NKH - 2, NKH - 1):
                        nc.tensor.matmul(
                            accs2[(bt, hb)][:],
                            s_act[:, kt, bt * 128:(bt + 1) * 128],
                            wB[:, kt, hb * 512:(hb + 1) * 512],
                            start=False, stop=(kt == NKH - 1))
                for hb in range(NHB):
                    acc = accs2[(bt, hb)]
                    hcol = slice(hb * 512, (hb + 1) * 512)
                    eng = nc.vector if hb % 2 == 0 else nc.gpsimd
                    t = scr.tile([128, 512], F32, tag="t")
                    nc.vector.tensor_add(t[:], acc[:], bn_bc[:, hcol])
                    eng.tensor_mul(t[:], t[:], s_omf[:, bt, hcol])
                    eng.tensor_add(t[:], t[:], s_fgx[:, bt, hcol])
                    nc.scalar.dma_start(
                        d_hy[bt * 128:(bt + 1) * 128, hb * 512:(hb + 1) * 512],
                        t[:])

    nc.finalize()
    # ldweights=False (skip the stationary reload when consecutive matmuls
    # share lhsT) raced nondeterministically on hardware in long streams —
    # wrong results in ~half of runs regardless of wait placement. Disable
    # it globally until the weight-slot semantics are understood.
    for blk in nc.m.functions[0].blocks:
        for inst in blk.instructions:
            if type(inst).__name__ == "InstMatmult" and inst.ldweights is False:
                inst.ldweights = None
    return nc


def _get_nc():
    global _NC_CACHE
    if _NC_CACHE is None:
        _NC_CACHE = build()
    return _NC_CACHE


def prepare_in_maps(input, hx, w_f, b_f, w_n, b_n):
    bf16 = ml_dtypes.bfloat16
    catT = np.ascontiguousarray(
        np.concatenate([hx, input], axis=1).T.astype(bf16))     # [H+I, B]
    hx16 = hx.astype(bf16)
    wfT = np.ascontiguousarray(w_f.T.astype(bf16))              # [H+I, H]
    wnT = np.ascontiguousarray(w_n.T.astype(bf16))
    bfr = np.ascontiguousarray(b_f[None, :].astype(np.float32))
    bnr = np.ascontiguousarray(b_n[None, :].astype(np.float32))
    in_maps = []
    for core in range(NCORES):
        cs = slice(core * BL, (core + 1) * BL)
        in_maps.append({
            "actT": np.ascontiguousarray(catT[:, cs]),
            "hxr": np.ascontiguousarray(hx16[cs, :]),
            "wfT": wfT,
            "wnT": wnT,
            "bf": bfr,
            "bn": bnr,
        })
    return in_maps


def assemble_output(results):
    return np.ascontiguousarray(np.concatenate(
        [np.asarray(results[c]["hy"], dtype=np.float32) for c in range(NCORES)],
        axis=0))


def kernel(input, hx, w_f, b_f, w_n, b_n, **_ignored):
    input = np.asarray(input, dtype=np.float32)
    hx = np.asarray(hx, dtype=np.float32)
    w_f = np.asarray(w_f, dtype=np.float32)
    b_f = np.asarray(b_f, dtype=np.float32)
    w_n = np.asarray(w_n, dtype=np.float32)
    b_n = np.asarray(b_n, dtype=np.float32)

    nc = _get_nc()
    in_maps = prepare_in_maps(input, hx, w_f, b_f, w_n, b_n)
    res = run_bass_kernel_spmd(nc, in_maps, list(range(NCORES)))
    return assemble_output(res.results)


if __name__ == "__main__":
    rng = np.random.default_rng(0)
    inputs = {
        "input": rng.uniform(-1, 1, (B, I)).astype(np.float32),
        "hx": rng.uniform(-1, 1, (B, H)).astype(np.float32),
        "w_f": (rng.standard_normal((H, H + I)) / np.sqrt(H + I)).astype(np.float32),
        "b_f": (rng.standard_normal(H) / np.sqrt(H + I)).astype(np.float32),
        "w_n": (rng.standard_normal((H, H + I)) / np.sqrt(H + I)).astype(np.float32),
        "b_n": (rng.standard_normal(H) / np.sqrt(H + I)).astype(np.float32),
    }
    out = kernel(**inputs)
    x64 = {k: v.astype(np.float64) for k, v in inputs.items()}
    cat = np.concatenate([x64["hx"], x64["input"]], axis=1)
    fg = (cat @ x64["w_f"].T + x64["b_f"] + 1.0) * 0.5
    fgx = fg * x64["hx"]
    ng = np.concatenate([fgx, x64["input"]], axis=1) @ x64["w_n"].T + x64["b_n"]
    exp = (1.0 - fg) * ng + fgx
    err = np.abs(out - exp).max() / np.abs(exp).max()
    print("rel err:", err)


# revision 10
# speedup vs baseline: 1.1587x; 1.0406x over previous
"""FSUMGU cell on 8 Trainium2 NeuronCores — pure data-parallel variant.

Each core owns 256 batch rows and computes the ENTIRE cell for them:
no collectives at all (the AllGather of the tensor-parallel variant
costs a ~40-70us CC-stream init barrier plus 2x ~30us serialized
gathers, a large fraction of which lands on the critical path).

Layout is [batch, hidden] ("row" orientation):
    zf[b, h] = sum_k catT[k, b].T @ wfT[k, h]   (stationary actT tile,
                                                 moving 512-wide weight cols)
    fg/omf/fgx elementwise on [128b, 512h] tiles (vector engine,
                                                  broadcast bias rows)
    fgx^T for GEMM2's hidden contraction comes from a DMA XBAR
    transpose (zero tensor-engine cost).

Weights stream through a single [128, 32, 2048] bf16 arena per matrix,
time-shared k-half by k-half: phase 1 consumes wf kh0 then kh1; wn's
input-half loads into wf-kh0's slot once that half is consumed, and
wn's fgx-half into wf-kh1's slot during phase 2's input-half.

Bias rows are broadcast to 128 partitions with ones-matmuls that
double as the tensor-engine HAM warm-up.

Every matmul self-loads its stationary tile: skipping the reload via
InstMatmult.ldweights=False (without a standalone InstLdweights) races
nondeterministically on hardware, so it is disabled globally by the
post-finalize pass at the end of build().
"""
import sys

sys.path.insert(0, "/opt/trn_rl_repo")

import numpy as np
import ml_dtypes
import concourse.tile as tile
from concourse import bacc, mybir
from concourse.bass_utils import run_bass_kernel_spmd

F32 = mybir.dt.float32
BF16 = mybir.dt.bfloat16
MULT = mybir.AluOpType.mult
ADD = mybir.AluOpType.add

B, H, I = 2048, 2048, 2048
NCORES = 8
BL = B // NCORES       # 256 batch rows per core
NBT = BL // 128        # 2 batch tiles
NHB = H // 512         # 4 hidden 512-col blocks
NKH = H // 128         # 16 k-tiles, hx/fgx half
NKI = I // 128         # 16 k-tiles, input half
NK = NKH + NKI         # 32
NWARM = 4              # pure warm-up matmuls before the bias broadcasts

_NC_CACHE = None


def build():
    nc = bacc.Bacc(None, target_bir_lowering=False, debug=False)
    d_actT = nc.dram_tensor("actT", [H + I, BL], BF16, kind="ExternalInput").ap()
    d_hxr = nc.dram_tensor("hxr", [BL, H], BF16, kind="ExternalInput").ap()
    d_wfT = nc.dram_tensor("wfT", [H + I, H], BF16, kind="ExternalInput").ap()
    d_wnT = nc.dram_tensor("wnT", [H + I, H], BF16, kind="ExternalInput").ap()
    d_bf = nc.dram_tensor("bf", [1, H], F32, kind="ExternalInput").ap()
    d_bn = nc.dram_tensor("bn", [1, H], F32, kind="ExternalInput").ap()
    d_hy = nc.dram_tensor("hy", [BL, H], F32, kind="ExternalOutput").ap()

    def kmaj(dram_ap, r0, nt, c0, ncols):
        """[nt*128, ncols] DRAM slab -> [128, nt, ncols] k-major AP."""
        return dram_ap[r0:r0 + nt * 128, c0:c0 + ncols].rearrange(
            "(t p) b -> p t b", p=128)

    with tile.TileContext(nc) as tc:
        with (
            tc.tile_pool(name="const", bufs=1) as const,
            tc.tile_pool(name="warena", bufs=1) as warena,
            tc.tile_pool(name="act", bufs=1) as actp,
            tc.tile_pool(name="pers", bufs=1) as pers,
            tc.tile_pool(name="fgt", bufs=3) as fgtp,
            tc.tile_pool(name="scr", bufs=3) as scr,
            tc.tile_pool(name="ps", bufs=8, space="PSUM") as ps,
        ):
            # ---- persistent SBUF tensors
            # weight arena: two k-half slots, time-shared wf -> wn
            wA = warena.tile([128, NKH, H], BF16, tag="wA")   # wf kh0 -> wn kh1(inp)
            wB = warena.tile([128, NKH, H], BF16, tag="wB")   # wf kh1 -> wn kh0(fgx)
            s_act = actp.tile([128, NK, BL], BF16, tag="actT")     # [hx; inp]^T
            s_hxr = pers.tile([128, NBT, H], BF16, tag="hxr")
            s_fgx = pers.tile([128, NBT, H], BF16, tag="fgx")
            s_omf = pers.tile([128, NBT, H], BF16, tag="omf")
            bias_bc = const.tile([128, 2, H], BF16, tag="biasbc")  # bfp, bn

            # ---- small loads (gpsimd): bias rows, hx row-layout copy
            rowf = const.tile([1, H], F32, tag="rowf")
            nc.gpsimd.dma_start(rowf[:], d_bf[:])
            nc.gpsimd.dma_start(s_hxr[:], d_hxr.rearrange("(t p) h -> p t h", p=128))

            # ---- bulk loads. sync ring: wf-kh0 x act-kh0 (kh0-critical),
            # then act-kh1, then its share of wn. scalar ring: wf-kh1 early
            # (needed only from kh1) + its share of wn + transposes + hy.
            # kt-pair interleave across BOTH HWDGE rings in strict need
            # order: [wA + act-kh0] then [wB + act-kh1]. Aggregate HBM BW
            # (~300 GB/s) is the limit; both rings must carry the critical
            # stream, earliest k-tiles first.
            for j in range(8):
                eng = nc.sync if j % 2 == 0 else nc.scalar
                kt = j * 2
                eng.dma_start(wA[:, kt:kt + 2, :], kmaj(d_wfT, kt * 128, 2, 0, H))
                eng.dma_start(s_act[:, kt:kt + 2, :],
                              kmaj(d_actT, kt * 128, 2, 0, BL))
            for j in range(8):
                eng = nc.sync if j % 2 == 0 else nc.scalar
                kt = j * 2
                eng.dma_start(s_act[:, 16 + kt:16 + kt + 2, :],
                              kmaj(d_actT, 2048 + kt * 128, 2, 0, BL))
                eng.dma_start(wB[:, kt:kt + 2, :],
                              kmaj(d_wfT, 2048 + kt * 128, 2, 0, H))

            # ---- bias prep: bfp=(b_f+1)/2 and bn as bf16 rows
            row16 = const.tile([1, 2, H], BF16, tag="row16")
            nc.vector.tensor_scalar(row16[:, 0, :], rowf[:],
                                    0.5, 0.5, MULT, ADD)
            nc.gpsimd.dma_start(rowf[:], d_bn[:])
            nc.vector.tensor_copy(row16[:, 1, :], rowf[:])
            ones = const.tile([1, 128], BF16, tag="ones")
            nc.vector.memset(ones[:], 1.0)
            wm = const.tile([128, 512], BF16, tag="wm")
            nc.vector.memset(wm[:], 0.0009765625)

            # ---- HAM warm-up + bias broadcast matmuls (also PE work)
            psw = ps.tile([128, 512], F32, tag="acc", name="psw")
            for i in range(NWARM):
                nc.tensor.matmul(psw[:], wm[:, :128], wm[:],
                                 start=(i == 0), stop=(i == NWARM - 1))
            for bi in range(2):
                for hb in range(NHB):
                    pb = ps.tile([128, 512], F32, tag="acc", name=f"pb{bi}{hb}")
                    nc.tensor.matmul(pb[:], ones[:],
                                     row16[:, bi, hb * 512:(hb + 1) * 512],
                                     start=True, stop=True)
                    nc.vector.tensor_copy(bias_bc[:, bi, hb * 512:(hb + 1) * 512],
                                          pb[:])
            bfp_bc = bias_bc[:, 0, :]
            bn_bc = bias_bc[:, 1, :]

            # ---- phase 1: zf -> fg/omf/fgx, k-half by k-half.
            # Stationary actT tile shared across the four h-blocks.
            accs1 = {}
            for bt in range(NBT):
                for hb in range(NHB):
                    accs1[(bt, hb)] = ps.tile([128, 512], F32, tag="acc",
                                              name=f"p1acc{bt}{hb}")
            for kh, warr in ((0, wA), (1, wB)):
                for bt in range(NBT):
                    for kt in range(NKH):
                        for hb in range(NHB):
                            mm = nc.tensor.matmul(
                                accs1[(bt, hb)][:],
                                s_act[:, kh * NKH + kt, bt * 128:(bt + 1) * 128],
                                warr[:, kt, hb * 512:(hb + 1) * 512],
                                start=(kh == 0 and kt == 0),
                                stop=(kh == 1 and kt == NKH - 1))
                            if hb > 0 and not (kh == 0 and kt == 0):
                                mm.ins.ldweights = False
            # ---- wn input-half loads into wf-kh0's slot (wA now dead)
            for j in range(8):
                eng = nc.sync if j % 2 == 0 else nc.scalar
                kt = j * 2
                eng.dma_start(wA[:, kt:kt + 2, :],
                              kmaj(d_wnT, 2048 + kt * 128, 2, 0, H))
            for bt in range(NBT):
                for hb in range(NHB):
                    acc = accs1[(bt, hb)]
                    hcol = slice(hb * 512, (hb + 1) * 512)
                    fgt = fgtp.tile([128, 512], BF16, tag="fgt")
                    nc.vector.scalar_tensor_tensor(
                        fgt[:], acc[:], 0.5, bfp_bc[:, hcol], MULT, ADD)
                    nc.vector.tensor_scalar(s_omf[:, bt, hcol], fgt[:],
                                            -1.0, 1.0, MULT, ADD)
                    nc.vector.tensor_mul(s_fgx[:, bt, hcol], fgt[:],
                                         s_hxr[:, bt, hcol])
                # fgx^T via DMA XBAR transposes (scalar HWDGE ring),
                # aliased into s_act's now-dead hx half. Per-k-tile so every
                # destination is a contiguous [128, 128] segment (XBAR
                # transpose corrupts non-contiguous destinations).
                for kt in range(NKH):
                    eng = nc.sync if kt % 2 == 0 else nc.scalar
                    eng.dma_start_transpose(
                        s_act[:, kt, bt * 128:(bt + 1) * 128],
                        s_fgx[:, bt, kt * 128:(kt + 1) * 128])

            # ---- wn fgx-half loads into wf-kh1's slot (wB dead after ph1)
            for j in range(8):
                eng = nc.sync if j % 2 == 0 else nc.scalar
                kt = j * 2
                eng.dma_start(wB[:, kt:kt + 2, :],
                              kmaj(d_wnT, kt * 128, 2, 0, H))

            # ---- phase 2: ng, hy. Input-half contraction first.
            accs2 = {}
            for bt in range(NBT):
                for hb in range(NHB):
                    accs2[(bt, hb)] = ps.tile([128, 512], F32, tag="acc",
                                              name=f"p2acc{bt}{hb}")
            for bt in range(NBT):
                for kt in range(NKI):
                    for hb in range(NHB):
                        mm = nc.tensor.matmul(
                            accs2[(bt, hb)][:],
                            s_act[:, NKH + kt, bt * 128:(bt + 1) * 128],
                            wA[:, kt, hb * 512:(hb + 1) * 512],
                            start=(kt == 0), stop=False)
                        if hb > 0 and kt > 0:
                            mm.ins.ldweights = False
            for bt in range(NBT):
                for kt in range(NKH - 2):
                    for hb in range(NHB):
                        mm = nc.tensor.matmul(
                            accs2[(bt, hb)][:],
                            s_act[:, kt, bt * 128:(bt + 1) * 128],
                            wB[:, kt, hb * 512:(hb + 1) * 512],
                            start=False, stop=False)
                        if hb > 0:
                            mm.ins.ldweights = False
                for hb in range(NHB):
                    for kt in (NKH - 2, NKH - 1):
                        nc.tensor.matmul(
                            accs2[(bt, hb)][:],
                            s_act[:, kt, bt * 128:(bt + 1) * 128],
                            wB[:, kt, hb * 512:(hb + 1) * 512],
                            start=False, stop=(kt == NKH - 1))
                for hb in range(NHB):
                    acc = accs2[(bt, hb)]
                    hcol = slice(hb * 512, (hb + 1) * 512)
                    eng = nc.vector if hb % 2 == 0 else nc.gpsimd
                    t = scr.tile([128, 512], F32, tag="t")
                    nc.vector.tensor_add(t[:], acc[:], bn_bc[:, hcol])
                    eng.tensor_mul(t[:], t[:], s_omf[:, bt, hcol])
                    eng.tensor_add(t[:], t[:], s_fgx[:, bt, hcol])
                    nc.scalar.dma_start(
                        d_hy[bt * 128:(bt + 1) * 128, hb * 512:(hb + 1) * 512],
                        t[:])

    nc.finalize()
    # ldweights=False (skip the stationary reload when consecutive matmuls
    # share lhsT) raced nondeterministically on hardware in long streams —
    # wrong results in ~half of runs regardless of wait placement. Disable
    # it globally until the weight-slot semantics are understood.
    for blk in nc.m.functions[0].blocks:
        for inst in blk.instructions:
            if type(inst).__name__ == "InstMatmult" and inst.ldweights is False:
                inst.ldweights = None
    return nc


def _get_nc():
    global _NC_CACHE
    if _NC_CACHE is None:
        _NC_CACHE = build()
    return _NC_CACHE


def prepare_in_maps(input, hx, w_f, b_f, w_n, b_n):
    bf16 = ml_dtypes.bfloat16
    catT = np.ascontiguousarray(
        np.concatenate([hx, input], axis=1).T.astype(bf16))     # [H+I, B]
    hx16 = hx.astype(bf16)
    wfT = np.ascontiguousarray(w_f.T.astype(bf16))              # [H+I, H]
    wnT = np.ascontiguousarray(w_n.T.astype(bf16))
    bfr = np.ascontiguousarray(b_f[None, :].astype(np.float32))
    bnr = np.ascontiguousarray(b_n[None, :].astype(np.float32))
    in_maps = []
    for core in range(NCORES):
        cs = slice(core * BL, (core + 1) * BL)
        in_maps.append({
            "actT": np.ascontiguousarray(catT[:, cs]),
            "hxr": np.ascontiguousarray(hx16[cs, :]),
            "wfT": wfT,
            "wnT": wnT,
            "bf": bfr,
            "bn": bnr,
        })
    return in_maps


def assemble_output(results):
    return np.ascontiguousarray(np.concatenate(
        [np.asarray(results[c]["hy"], dtype=np.float32) for c in range(NCORES)],
        axis=0))


def kernel(input, hx, w_f, b_f, w_n, b_n, **_ignored):
    input = np.asarray(input, dtype=np.float32)
    hx = np.asarray(hx, dtype=np.float32)
    w_f = np.asarray(w_f, dtype=np.float32)
    b_f = np.asarray(b_f, dtype=np.float32)
    w_n = np.asarray(w_n, dtype=np.float32)
    b_n = np.asarray(b_n, dtype=np.float32)

    nc = _get_nc()
    in_maps = prepare_in_maps(input, hx, w_f, b_f, w_n, b_n)
    res = run_bass_kernel_spmd(nc, in_maps, list(range(NCORES)))
    return assemble_output(res.results)


if __name__ == "__main__":
    rng = np.random.default_rng(0)
    inputs = {
        "input": rng.uniform(-1, 1, (B, I)).astype(np.float32),
        "hx": rng.uniform(-1, 1, (B, H)).astype(np.float32),
        "w_f": (rng.standard_normal((H, H + I)) / np.sqrt(H + I)).astype(np.float32),
        "b_f": (rng.standard_normal(H) / np.sqrt(H + I)).astype(np.float32),
        "w_n": (rng.standard_normal((H, H + I)) / np.sqrt(H + I)).astype(np.float32),
        "b_n": (rng.standard_normal(H) / np.sqrt(H + I)).astype(np.float32),
    }
    out = kernel(**inputs)
    x64 = {k: v.astype(np.float64) for k, v in inputs.items()}
    cat = np.concatenate([x64["hx"], x64["input"]], axis=1)
    fg = (cat @ x64["w_f"].T + x64["b_f"] + 1.0) * 0.5
    fgx = fg * x64["hx"]
    ng = np.concatenate([fgx, x64["input"]], axis=1) @ x64["w_n"].T + x64["b_n"]
    exp = (1.0 - fg) * ng + fgx
    err = np.abs(out - exp).max() / np.abs(exp).max()
    print("rel err:", err)


# revision 11
# speedup vs baseline: 1.3305x; 1.1483x over previous
"""FSUMGU cell on 8 Trainium2 NeuronCores — pure data-parallel variant.

Each core owns 256 batch rows and computes the ENTIRE cell for them:
no collectives at all (the AllGather of the tensor-parallel variant
costs a ~40-70us CC-stream init barrier plus 2x ~30us serialized
gathers, a large fraction of which lands on the critical path).

Layout is [batch, hidden] ("row" orientation):
    zf[b, h] = sum_k catT[k, b].T @ wfT[k, h]   (stationary actT tile,
                                                 moving 512-wide weight cols)
    fg/omf/fgx elementwise on [128b, 512h] tiles (vector engine,
                                                  broadcast bias rows)
    fgx^T for GEMM2's hidden contraction comes from a DMA XBAR
    transpose (zero tensor-engine cost).

Weights stream through a single [128, 32, 2048] bf16 arena per matrix,
time-shared k-half by k-half: phase 1 consumes wf kh0 then kh1; wn's
input-half loads into wf-kh0's slot once that half is consumed, and
wn's fgx-half into wf-kh1's slot during phase 2's input-half.

Bias rows are broadcast to 128 partitions with ones-matmuls that
double as the tensor-engine HAM warm-up.

Every matmul self-loads its stationary tile: skipping the reload via
InstMatmult.ldweights=False (without a standalone InstLdweights) races
nondeterministically on hardware, so it is disabled globally by the
post-finalize pass at the end of build().
"""
import sys

sys.path.insert(0, "/opt/trn_rl_repo")

import numpy as np
import ml_dtypes
import concourse.tile as tile
from concourse import bacc, mybir
from concourse.bass_utils import run_bass_kernel_spmd

F32 = mybir.dt.float32
BF16 = mybir.dt.bfloat16
MULT = mybir.AluOpType.mult
ADD = mybir.AluOpType.add

B, H, I = 2048, 2048, 2048
NCORES = 8
BL = B // NCORES       # 256 batch rows per core
NBT = BL // 128        # 2 batch tiles
NHB = H // 512         # 4 hidden 512-col blocks
NKH = H // 128         # 16 k-tiles, hx/fgx half
NKI = I // 128         # 16 k-tiles, input half
NK = NKH + NKI         # 32
NWARM = 4              # pure warm-up matmuls before the bias broadcasts

_NC_CACHE = None


def build():
    nc = bacc.Bacc(None, target_bir_lowering=False, debug=False)
    d_actT = nc.dram_tensor("actT", [H + I, BL], BF16, kind="ExternalInput").ap()
    d_hxr = nc.dram_tensor("hxr", [BL, H], BF16, kind="ExternalInput").ap()
    d_wfT = nc.dram_tensor("wfT", [H + I, H], BF16, kind="ExternalInput").ap()
    d_wnT = nc.dram_tensor("wnT", [H + I, H], BF16, kind="ExternalInput").ap()
    d_bf = nc.dram_tensor("bf", [1, H], F32, kind="ExternalInput").ap()
    d_bn = nc.dram_tensor("bn", [1, H], F32, kind="ExternalInput").ap()
    d_hy = nc.dram_tensor("hy", [BL, H], F32, kind="ExternalOutput").ap()

    def kmaj(dram_ap, r0, nt, c0, ncols):
        """[nt*128, ncols] DRAM slab -> [128, nt, ncols] k-major AP."""
        return dram_ap[r0:r0 + nt * 128, c0:c0 + ncols].rearrange(
            "(t p) b -> p t b", p=128)

    with tile.TileContext(nc) as tc:
        with (
            tc.tile_pool(name="const", bufs=1) as const,
            tc.tile_pool(name="warena", bufs=1) as warena,
            tc.tile_pool(name="act", bufs=1) as actp,
            tc.tile_pool(name="pers", bufs=1) as pers,
            tc.tile_pool(name="fgt", bufs=3) as fgtp,
            tc.tile_pool(name="scr", bufs=3) as scr,
            tc.tile_pool(name="ps", bufs=8, space="PSUM") as ps,
        ):
            # ---- persistent SBUF tensors
            # weight arena: two k-half slots, time-shared wf -> wn
            wA = warena.tile([128, NKH, H], BF16, tag="wA")   # wf kh0 -> wn kh1(inp)
            wB = warena.tile([128, NKH, H], BF16, tag="wB")   # wf kh1 -> wn kh0(fgx)
            s_act = actp.tile([128, NK, BL], BF16, tag="actT")     # [hx; inp]^T
            s_hxr = pers.tile([128, NBT, H], BF16, tag="hxr")
            # fgx^T lives in s_hxr's memory (tag reuse): each bt half is only
            # written after that bt's fgx elementwise product has consumed
            # the corresponding hx rows. Contiguous per-bt slab -> safe XBAR
            # transpose destination.
            s_fgxT = pers.tile([128, NBT, H], BF16, tag="hxr", name="s_fgxT")
            s_fgx = pers.tile([128, NBT, H], BF16, tag="fgx")
            s_omf = pers.tile([128, NBT, H], BF16, tag="omf")
            bias_bc = const.tile([128, 2, H], BF16, tag="biasbc")  # bfp, bn

            # ---- small loads (gpsimd): bias rows, hx row-layout copy
            rowf = const.tile([1, H], F32, tag="rowf")
            nc.gpsimd.dma_start(rowf[:], d_bf[:])
            nc.gpsimd.dma_start(s_hxr[:], d_hxr.rearrange("(t p) h -> p t h", p=128))

            # ---- bulk loads. sync ring: wf-kh0 x act-kh0 (kh0-critical),
            # then act-kh1, then its share of wn. scalar ring: wf-kh1 early
            # (needed only from kh1) + its share of wn + transposes + hy.
            # kt-pair interleave across BOTH HWDGE rings in strict need
            # order: [wA + act-kh0] then [wB + act-kh1]. Aggregate HBM BW
            # (~300 GB/s) is the limit; both rings must carry the critical
            # stream, earliest k-tiles first.
            for j in range(8):
                eng = nc.sync if j % 2 == 0 else nc.scalar
                kt = j * 2
                eng.dma_start(wA[:, kt:kt + 2, :], kmaj(d_wfT, kt * 128, 2, 0, H))
                eng.dma_start(s_act[:, kt:kt + 2, :],
                              kmaj(d_actT, kt * 128, 2, 0, BL))
            for j in range(8):
                eng = nc.sync if j % 2 == 0 else nc.scalar
                kt = j * 2
                eng.dma_start(s_act[:, 16 + kt:16 + kt + 2, :],
                              kmaj(d_actT, 2048 + kt * 128, 2, 0, BL))
                eng.dma_start(wB[:, kt:kt + 2, :],
                              kmaj(d_wfT, 2048 + kt * 128, 2, 0, H))

            # ---- bias prep: bfp=(b_f+1)/2 and bn as bf16 rows
            row16 = const.tile([1, 2, H], BF16, tag="row16")
            nc.vector.tensor_scalar(row16[:, 0, :], rowf[:],
                                    0.5, 0.5, MULT, ADD)
            nc.gpsimd.dma_start(rowf[:], d_bn[:])
            nc.vector.tensor_copy(row16[:, 1, :], rowf[:])
            ones = const.tile([1, 128], BF16, tag="ones")
            nc.vector.memset(ones[:], 1.0)
            wm = const.tile([128, 512], BF16, tag="wm")
            nc.vector.memset(wm[:], 0.0009765625)

            # ---- HAM warm-up + bias broadcast matmuls (also PE work)
            psw = ps.tile([128, 512], F32, tag="acc", name="psw")
            for i in range(NWARM):
                nc.tensor.matmul(psw[:], wm[:, :128], wm[:],
                                 start=(i == 0), stop=(i == NWARM - 1))
            for bi in range(2):
                for hb in range(NHB):
                    pb = ps.tile([128, 512], F32, tag="acc", name=f"pb{bi}{hb}")
                    nc.tensor.matmul(pb[:], ones[:],
                                     row16[:, bi, hb * 512:(hb + 1) * 512],
                                     start=True, stop=True)
                    nc.vector.tensor_copy(bias_bc[:, bi, hb * 512:(hb + 1) * 512],
                                          pb[:])
            bfp_bc = bias_bc[:, 0, :]
            bn_bc = bias_bc[:, 1, :]

            # ---- phase 1: zf -> fg/omf/fgx, k-half by k-half.
            # Stationary actT tile shared across the four h-blocks.
            accs1 = {}
            for bt in range(NBT):
                for hb in range(NHB):
                    accs1[(bt, hb)] = ps.tile([128, 512], F32, tag="acc",
                                              name=f"p1acc{bt}{hb}")
            for kh, warr in ((0, wA), (1, wB)):
                for bt in range(NBT):
                    for kt in range(NKH):
                        for hb in range(NHB):
                            mm = nc.tensor.matmul(
                                accs1[(bt, hb)][:],
                                s_act[:, kh * NKH + kt, bt * 128:(bt + 1) * 128],
                                warr[:, kt, hb * 512:(hb + 1) * 512],
                                start=(kh == 0 and kt == 0),
                                stop=(kh == 1 and kt == NKH - 1))
                            if hb > 0 and not (kh == 0 and kt == 0):
                                mm.ins.ldweights = False
            # ---- wn input-half loads into wf-kh0's slot (wA now dead)
            for j in range(8):
                eng = nc.sync if j % 2 == 0 else nc.scalar
                kt = j * 2
                eng.dma_start(wA[:, kt:kt + 2, :],
                              kmaj(d_wnT, 2048 + kt * 128, 2, 0, H))
            for bt in range(NBT):
                for hb in range(NHB):
                    acc = accs1[(bt, hb)]
                    hcol = slice(hb * 512, (hb + 1) * 512)
                    fgt = fgtp.tile([128, 512], BF16, tag="fgt")
                    nc.vector.scalar_tensor_tensor(
                        fgt[:], acc[:], 0.5, bfp_bc[:, hcol], MULT, ADD)
                    nc.vector.tensor_scalar(s_omf[:, bt, hcol], fgt[:],
                                            -1.0, 1.0, MULT, ADD)
                    nc.vector.tensor_mul(s_fgx[:, bt, hcol], fgt[:],
                                         s_hxr[:, bt, hcol])
                # fgx^T via ONE whole-slab DMA XBAR transpose per b-tile
                # (contiguous destination -> safe; and only ~2us of ring
                # time each, so the wn reload DMAs behind them aren't
                # starved the way 32 per-tile transposes starved them).
                eng = nc.scalar if bt % 2 == 0 else nc.sync
                eng.dma_start_transpose(
                    s_fgxT[:, bt, :].rearrange("p (t b) -> p t b", b=128),
                    s_fgx[:, bt, :])

            # ---- wn fgx-half loads into wf-kh1's slot (wB dead after ph1)
            for j in range(8):
                eng = nc.sync if j % 2 == 0 else nc.scalar
                kt = j * 2
                eng.dma_start(wB[:, kt:kt + 2, :],
                              kmaj(d_wnT, kt * 128, 2, 0, H))

            # ---- phase 2: ng, hy. Input-half contraction first.
            accs2 = {}
            for bt in range(NBT):
                for hb in range(NHB):
                    accs2[(bt, hb)] = ps.tile([128, 512], F32, tag="acc",
                                              name=f"p2acc{bt}{hb}")
            for bt in range(NBT):
                for kt in range(NKI):
                    for hb in range(NHB):
                        mm = nc.tensor.matmul(
                            accs2[(bt, hb)][:],
                            s_act[:, NKH + kt, bt * 128:(bt + 1) * 128],
                            wA[:, kt, hb * 512:(hb + 1) * 512],
                            start=(kt == 0), stop=False)
                        if hb > 0 and kt > 0:
                            mm.ins.ldweights = False
            for bt in range(NBT):
                fT = s_fgxT[:, bt, :].rearrange("p (t b) -> p t b", b=128)
                for kt in range(NKH - 2):
                    for hb in range(NHB):
                        mm = nc.tensor.matmul(
                            accs2[(bt, hb)][:],
                            fT[:, kt, :],
                            wB[:, kt, hb * 512:(hb + 1) * 512],
                            start=False, stop=False)
                        if hb > 0:
                            mm.ins.ldweights = False
                for hb in range(NHB):
                    for kt in (NKH - 2, NKH - 1):
                        nc.tensor.matmul(
                            accs2[(bt, hb)][:],
                            fT[:, kt, :],
                            wB[:, kt, hb * 512:(hb + 1) * 512],
                            start=False, stop=(kt == NKH - 1))
                for hb in range(NHB):
                    acc = accs2[(bt, hb)]
                    hcol = slice(hb * 512, (hb + 1) * 512)
                    eng = nc.vector if hb % 2 == 0 else nc.gpsimd
                    t = scr.tile([128, 512], F32, tag="t")
                    nc.vector.tensor_add(t[:], acc[:], bn_bc[:, hcol])
                    eng.tensor_mul(t[:], t[:], s_omf[:, bt, hcol])
                    eng.tensor_add(t[:], t[:], s_fgx[:, bt, hcol])
                    nc.scalar.dma_start(
                        d_hy[bt * 128:(bt + 1) * 128, hb * 512:(hb + 1) * 512],
                        t[:])

    nc.finalize()
    # ldweights=False (skip the stationary reload when consecutive matmuls
    # share lhsT) raced nondeterministically on hardware in long streams —
    # wrong results in ~half of runs regardless of wait placement. Disable
    # it globally until the weight-slot semantics are understood.
    for blk in nc.m.functions[0].blocks:
        for inst in blk.instructions:
            if type(inst).__name__ == "InstMatmult" and inst.ldweights is False:
                inst.ldweights = None
    return nc


def _get_nc():
    global _NC_CACHE
    if _NC_CACHE is None:
        _NC_CACHE = build()
    return _NC_CACHE


def prepare_in_maps(input, hx, w_f, b_f, w_n, b_n):
    bf16 = ml_dtypes.bfloat16
    catT = np.ascontiguousarray(
        np.concatenate([hx, input], axis=1).T.astype(bf16))     # [H+I, B]
    hx16 = hx.astype(bf16)
    wfT = np.ascontiguousarray(w_f.T.astype(bf16))              # [H+I, H]
    wnT = np.ascontiguousarray(w_n.T.astype(bf16))
    bfr = np.ascontiguousarray(b_f[None, :].astype(np.float32))
    bnr = np.ascontiguousarray(b_n[None, :].astype(np.float32))
    in_maps = []
    for core in range(NCORES):
        cs = slice(core * BL, (core + 1) * BL)
        in_maps.append({
            "actT": np.ascontiguousarray(catT[:, cs]),
            "hxr": np.ascontiguousarray(hx16[cs, :]),
            "wfT": wfT,
            "wnT": wnT,
            "bf": bfr,
            "bn": bnr,
        })
    return in_maps


def assemble_output(results):
    return np.ascontiguousarray(np.concatenate(
        [np.asarray(results[c]["hy"], dtype=np.float32) for c in range(NCORES)],
        axis=0))


def kernel(input, hx, w_f, b_f, w_n, b_n, **_ignored):
    input = np.asarray(input, dtype=np.float32)
    hx = np.asarray(hx, dtype=np.float32)
    w_f = np.asarray(w_f, dtype=np.float32)
    b_f = np.asarray(b_f, dtype=np.float32)
    w_n = np.asarray(w_n, dtype=np.float32)
    b_n = np.asarray(b_n, dtype=np.float32)

    nc = _get_nc()
    in_maps = prepare_in_maps(input, hx, w_f, b_f, w_n, b_n)
    res = run_bass_kernel_spmd(nc, in_maps, list(range(NCORES)))
    return assemble_output(res.results)


if __name__ == "__main__":
    rng = np.random.default_rng(0)
    inputs = {
        "input": rng.uniform(-1, 1, (B, I)).astype(np.float32),
        "hx": rng.uniform(-1, 1, (B, H)).astype(np.float32),
        "w_f": (rng.standard_normal((H, H + I)) / np.sqrt(H + I)).astype(np.float32),
        "b_f": (rng.standard_normal(H) / np.sqrt(H + I)).astype(np.float32),
        "w_n": (rng.standard_normal((H, H + I)) / np.sqrt(H + I)).astype(np.float32),
        "b_n": (rng.standard_normal(H) / np.sqrt(H + I)).astype(np.float32),
    }
    out = kernel(**inputs)
    x64 = {k: v.astype(np.float64) for k, v in inputs.items()}
    cat = np.concatenate([x64["hx"], x64["input"]], axis=1)
    fg = (cat @ x64["w_f"].T + x64["b_f"] + 1.0) * 0.5
    fgx = fg * x64["hx"]
    ng = np.concatenate([fgx, x64["input"]], axis=1) @ x64["w_n"].T + x64["b_n"]
    exp = (1.0 - fg) * ng + fgx
    err = np.abs(out - exp).max() / np.abs(exp).max()
    print("rel err:", err)
